# revision 71
# baseline (speedup 1.0000x reference)
"""BinConv2dEval Trainium2 kernel (fp8 DoubleRow, packed-65 layout).

y = conv2d(x, W, stride 1, pad 1) + bias ; out = (round(y) * sign >= 0) ? 1 : 0

All values are integers (x in {0,1}, W in {-1,0,1}, bias integer), so round()
is a no-op and everything is exact in fp8e4 matmuls with fp32 PSUM
accumulation. Folding: with s = sign[c] in {+-1},
    (conv + bias) * s >= 0   <=>   conv(x, s*W) >= -s*bias
so host-side we fold sign into the (still ternary) weights and compare each
output channel against a per-channel threshold with one DVE is_ge op.

Sharding: data-parallel over batch N=32 -> 4 images per core on 8 cores.
Weights/bias/sign are tiny and replicated.

Layout: width-65 rows ([64 data][0]) with SHARED zero rows between images:
row stream = [z, img0 r0..63, z, img1 r0..63, z, img2 ..., z, img3 ..., z]
= 261 rows x 65 = 16965 elems per partition. The single trailing zero col
doubles as both x(r,-1) of the next row and x(r,64) of its own, and each
separator zero row is both bottom pad of img i and top pad of img i+1. The
conv's 9 taps become pure element offsets (kh-1)*65 + (kw-1), and the
4-image output region is ONE contiguous stream of 16834 positions per cout
half (junk at stored col 64 and in separator rows; host strips ~2.7%).

Taps pair into fp8 DoubleRow matmuls (2 MACs/cell/cycle, contraction 256):
(kh=0,kw)+(kh=1,kw) at stride 65 for kw in 0..2, (kh=2,kw=0)+(kh=2,kw=2) at
stride 2, and the odd 9th tap (kh=2,kw=1) rides a 5th DR pass whose slot-0
weights are ZERO (the dummy rhs row points at in-bounds x) — every pass runs
at the 0.5 cycles/row DR rate, 2.5 cycles/elem/half vs 3.0 with a
normal-rate single-tap pass.
Per half: 33 PSUM tiles (32x512 + 450), weight-stationary spans of 4 tiles
(4 active + 4 draining = exactly the 8 PSUM banks). Drains alternate
between DVE tensor_scalar(is_ge) and ACT activation(Sigmoid, scale=64,
bias=32-64*th) — exactly 1.0/0.0 for integer args — writing 0/1 **fp8** to
SBUF (4x less output DMA than fp32; host upcasts). The last two half-1
output DMAs ride the sync ring so the final one finds a warm ring and the
sync ring's fast (~0.4us) completion-sem delivery.

Head: weights+x ride ONE packed dram tensor; the critical first chunk
(pair-4+pair-0 weights, guard, tile-0/1 x) is split across THREE DMA
rings — sync HWDGE (weights), gpsimd SWDGE (tile-0 x), scalar HWDGE
(tile-1 x) — to cut time-to-first-data to ~10.1us. Zero-weight warm-up
matmuls bridge the fixed ~7.2us engine preamble to the chunk-0 semaphore
(~10.4us) and deliberately overshoot: a PE idle gap there resets the HAM
un-throttle ramp (~4us penalty on a slow-DMA run). Measured stream sits at
the systolic floor (~220ns per 512-col DR matmul = ~97% of the 157 TF/s
fp8 peak; note fp8 DR is 2 MAC/cell/cycle on real HW — the cost model's
0.5 cycles/row is 2x optimistic). Remaining exec time is the fixed
preamble, a fixed ~11us-period ~440ns stall (i-fetch/refresh, ~0.6% of
stream), the tail DMA round-trip, and the NEFF semaphore-teardown
epilogue.
"""

import numpy as np
import ml_dtypes

N, CIN, H, W = 32, 128, 64, 64
COUT, KH, KW = 256, 3, 3
N_CORES = 8
IMGS = N // N_CORES          # 4 images per core
WS = W + 1                   # 65: stored row width ([64 data][0])
ROWS_T = 1 + IMGS * (H + 1)  # 261 stored rows (shared separators)
XD = ROWS_T * WS             # 16965 elements per partition
GF = 16                      # guard zeros before the image block
XTOT = GF + XD
OBASE = WS                   # first output position in the stream (img0 r0 c0)
OUT_N = (1 + IMGS * (H + 1) - 2) * WS + (W - 1) - OBASE + 1  # 16834
NB = 512                     # full PSUM tile free dim (one bank)
NBS = [NB] * 32 + [OUT_N - 32 * NB]   # 33 tiles: 32x512 + 450
NHALF = COUT // 128          # 2 cout halves
NPAIR = 5                    # DoubleRow passes (pair 4 = single tap + zeros)
FP8 = ml_dtypes.float8_e4m3  # TRN float8e4; {-1,0,1} and {0,1} are exact
NWARM = 8                    # zero-weight PE warm-up matmuls (N=512, cold)
                             # deliberately OVERSHOOTS the typical chunk-0
                             # DMA semaphore (~10.4us; warm-ups end ~11.4):
                             # a PE idle gap before the stream resets the
                             # HAM un-throttle ramp (~4us loss on a slow DMA
                             # run), while overshoot costs ~0.4us/warm-up
                             # only when the DMA is early

# weight-stationary spans (start tile, n tiles). 4-tile steady-state
# spans: 4 active + 4 draining = exactly the 8 PSUM banks, so every span
# boundary finds its banks already free. Half-0 ramps 1/2/2 so compute can
# start on a small first DMA chunk; half-1 ends on a lone 450-col tile so
# the final drain+DMA is minimal.
SG_HALF = (
    ((0, 1), (1, 2), (3, 2), (5, 4), (9, 4), (13, 4), (17, 4), (21, 4),
     (25, 4), (29, 4)),
    ((0, 4), (4, 4), (8, 4), (12, 4), (16, 4), (20, 4), (24, 4), (28, 2),
     (30, 2), (32, 1)),
)
N_TAIL_SG = 3  # last half-1 subgroups whose output DMAs ride the sync ring
               # (2+2+1 tiles: each small DMA issues as soon as its own two
               # drains finish, so the transfers overlap the remaining
               # drains instead of one 262KB DMA gating the 58KB final)

# The fp8 weights and padded x are packed into ONE dram tensor
# [wtp pair4 512 | wtp pair0 512 | guard 16 | xpad 16965 | wtp pairs1-3
# 1536]. Pair 4 = (zeros, (kh2,kw1)): the 9th tap's DR partner slot; pass
# order is (4, 0, 1, 2, 3), so the critical first chunk carries pair-4 AND
# pair-0 weights plus x tiles 0-1. It is split into two column strips on
# TWO rings (sync + scalar, the only HWDGE engines): one's descriptors ramp the shared DMA
# engines slowly (~54 GB/s in the first us), two rings halve the
# time-to-first-matmul-data. The pair-1..3 weights follow on the sync ring
# (needed two passes later).
W4_OFF = 0                   # pair-4 weights [0 256 | w(kh2,kw1) 256]
W0_OFF = 2 * COUT            # pair-0 weights [w0 256 | w1 256]
XS_OFF = 4 * COUT + GF       # 1040: xpad position in the packed tensor
WR_OFF = XS_OFF + XD         # pair 1..3 weights
XWTOT = WR_OFF + (NPAIR - 2) * 2 * COUT
XMID = XS_OFF + 643          # 1683: gpsimd/scalar boundary inside chunk 0
# chunk regions (packed-tensor col ranges) in need order. Tile t reads
# xpad [OBASE + 512t - 66, OBASE + 512t + NBS + 65]. The sync ring's
# completion semaphore lands ~0.4us after its data; the scalar ring's
# trickles in over ~1.2us — so sync carries the weights prefix, the
# gpsimd SWDGE ring carries tile-0 x (both first-DR-critical, in
# parallel), and scalar carries tile-1 x, consumed ~5 passes later (the
# first subgroup is a single tile), where the trickle doesn't bite. The
# pairs-1..3 chunk follows on sync, arriving ~one pass ahead of its first
# consumer in a typical run.
XCH0 = (
    (0, XS_OFF),                 # weights + guard: sync
    (XS_OFF, XMID),              # tile-0 x: gpsimd
    (XMID, XS_OFF + 1170),       # tile-1 x: scalar
)
XCH = (
    (WR_OFF, XWTOT),             # pair 1..3 weights
    (XS_OFF + 1170, XS_OFF + 2210),   # x tiles 2-3
    (XS_OFF + 2210, XS_OFF + 8450),   # x tiles 4-15
    (XS_OFF + 8450, WR_OFF),          # x rest
)

_CACHE = {}
LAST_RESULT = None           # BassKernelResults of the last run (for profiling)


def _build():
    import concourse.bass as bass
    import concourse.mybir as mybir
    from concourse import bacc
    from concourse.tile import TileContext

    dt = mybir.dt
    nc = bacc.Bacc()
    # packed per layout above; pair weights are [cin, pair, 2, cout] blocks
    # (pairs 0..2 = (kh0,kh1) per kw, pair 3 = ((kh2,kw0),(kh2,kw2)), pair 4
    # = (zeros, (kh2,kw1)))
    xw = nc.dram_tensor("xw", [128, XWTOT], dt.float8e4, kind="ExternalInput")
    # [th | thb]: DVE-drain thresholds and the ACT-engine drain bias
    # 32 - 64*th (sigmoid(64*conv + thb) saturates to exactly 1.0 for
    # arg >= +32 / 0.0 for arg <= -32, and integer conv, th keep the arg
    # outside (-32, 32)); one tensor = one DMA = 128 descriptors, not 256
    th2 = nc.dram_tensor("th2", [128, 2 * NHALF], dt.float32, kind="ExternalInput")
    out = nc.dram_tensor(
        "out", [NHALF, 128, OUT_N], dt.float8e4, kind="ExternalOutput"
    )

    DR = mybir.MatmulPerfMode.DoubleRow
    # (pair rhs offset, pair stride) per DoubleRow pair index; pair 4's
    # slot-0 row (offset 0, the (1,1) tap position — always in-bounds) is a
    # dummy multiplied by zero weights, slot 1 is the real (kh2,kw1) tap
    PAIR_GEOM = [(-66, WS), (-65, WS), (-64, WS), (64, 2), (0, WS)]

    with TileContext(nc) as tc:
        with (
            tc.tile_pool(name="const", bufs=1) as cpool,
            tc.tile_pool(name="xin", bufs=1) as xpool,
            tc.tile_pool(name="psum", bufs=8, space="PSUM") as ppool,
            tc.tile_pool(name="outb", bufs=5) as opool,
        ):
            # warm-up operand first in gpsimd order so dummies start early
            # (all engines are gated by the ~7.2us preamble barrier; gpsimd
            # dispatches its first memset soonest after it). The tile is
            # only 128 wide — a 202ns memset — and serves as warm-up lhs
            # AND rhs via a stride-0 free-dim AP (512 reads of column 0).
            wz_t = cpool.tile([128, 128], dt.float8e4, tag="wz")
            nc.gpsimd.memset(wz_t[:], 0)

            xw_t = xpool.tile([128, XWTOT], dt.float8e4, tag="xw")
            xs = xw_t[:]
            # (the host-packed tensor already holds the zero guard cols, so
            # junk reads at o=OBASE-66 can't hit fp8 NaNs)

            # chunk 0 splits across the sync + scalar rings (time-to-first-
            # data); the pair-1/2/3 weight chunks follow on sync, x tiles
            # 2-4 on scalar, the thresholds after pair 3 (needed by the
            # first drain, ~2us later)
            th2_t = cpool.tile([128, 2 * NHALF], dt.float32, tag="th2")
            for eng, (lo, hi) in zip(
                (nc.sync, nc.gpsimd, nc.scalar), XCH0
            ):
                eng.dma_start(out=xs[:, lo:hi], in_=xw[:, lo:hi])
            for i, (lo, hi) in enumerate(XCH):
                nc.sync.dma_start(out=xs[:, lo:hi], in_=xw[:, lo:hi])
                if i == 1:
                    nc.sync.dma_start(out=th2_t[:], in_=th2[:])

            # Warm the PE clock (HAM un-throttle needs a few us of sustained
            # activity) with zero-weight matmuls on a zeroed scratch tile
            # while the input DMA doorbell+transfer is still in flight.
            pd = ppool.tile([128, NB], dt.float32, tag="ps", name="pd")
            wz = wz_t[:]
            wz_rhs = bass.AP(wz.tensor, 0, [list(wz.ap[0]), [0, NB]])
            for _ in range(NWARM):
                nc.tensor.matmul(
                    pd[:], wz[:, :128], wz_rhs, start=True, stop=True
                )

            xten, xap0 = xs.tensor, list(xs.ap[0])

            def rhs_pair(base, p, nb):
                off, stride = PAIR_GEOM[p]
                return bass.AP(xten, base + off, [xap0, [stride, 2], [1, nb]])

            def lhs_pair(p, h):
                if p == 4:
                    base = W4_OFF
                elif p == 0:
                    base = W0_OFF
                else:
                    base = WR_OFF + (p - 1) * 2 * COUT
                return bass.AP(
                    xten, base + h * 128, [xap0, [COUT, 2], [1, 128]]
                )

            starts = [NB * t for t in range(len(NBS))]
            for h in range(NHALF):
                for sg_i, (sg_start, sg_n) in enumerate(SG_HALF[h]):
                    # the last TWO half-1 subgroups' output DMAs ride the
                    # SYNC ring: the (30,2) DMA issues while (32,1) still
                    # computes (re-warming the ring's doorbell path), and
                    # the final 450-col DMA then lands with a hot ring + the
                    # sync ring's fast (~0.4us) completion-sem delivery —
                    # the scalar ring's sem trickles in over ~1.2us and the
                    # end barrier waits on it. (Routing MORE half-1 DMAs to
                    # sync backs up the ring and delays the tail instead.)
                    tail_i = len(SG_HALF[1]) - N_TAIL_SG
                    is_tail = h == 1 and sg_i >= tail_i
                    oq = nc.sync if h == 0 or is_tail else nc.scalar
                    tls = list(range(sg_start, sg_start + sg_n))
                    ow = sum(NBS[t] for t in tls)
                    ot = opool.tile([128, ow], dt.float8e4, tag="ot", name="ot")
                    ps = [
                        ppool.tile([128, NBS[t]], dt.float32, tag="ps", name="ps")
                        for t in tls
                    ]
                    # pair 4 (the zero-padded 9th tap) runs FIRST with the
                    # start flag; pair 3 carries stop — a DR matmul with
                    # acc stop + non-16-aligned rhs pair stride hangs the PE
                    # (pairs 0..3 with start/mid flags are field-proven, and
                    # tile_matmul's DR+stop uses 16-aligned strides)
                    for k, p in enumerate((4, 0, 1, 2, 3)):
                        st, sp = k == 0, k == NPAIR - 1
                        wap = lhs_pair(p, h)
                        for j, t in enumerate(tls):
                            nc.tensor.matmul(
                                ps[j][:],
                                wap,
                                rhs_pair(XS_OFF + OBASE + starts[t], p, NBS[t]),
                                perf_mode=DR,
                                start=st,
                                stop=sp,
                            )
                    ob = 0
                    for j, t in enumerate(tls):
                        if h == 1 and sg_i == len(SG_HALF[1]) - 1:
                            # very last tile: drain both halves in PARALLEL
                            # on DVE + ACT (~0.38us vs 0.69us DVE-only) —
                            # with the 2+2+1 tail the ACT engine's previous
                            # drain ends BEFORE the last matmul, so unlike
                            # the old 4-tile tail it no longer starts late
                            hw2 = NBS[t] // 2
                            nc.vector.tensor_scalar(
                                out=ot[:, ob : ob + hw2],
                                in0=ps[j][:, :hw2],
                                scalar1=th2_t[:, h : h + 1],
                                scalar2=None,
                                op0=mybir.AluOpType.is_ge,
                            )
                            nc.scalar.activation(
                                out=ot[:, ob + hw2 : ob + NBS[t]],
                                in_=ps[j][:, hw2:],
                                func=mybir.ActivationFunctionType.Sigmoid,
                                bias=th2_t[:, NHALF + h : NHALF + h + 1],
                                scale=64.0,
                            )
                        elif j % 2 == 0:
                            nc.vector.tensor_scalar(
                                out=ot[:, ob : ob + NBS[t]],
                                in0=ps[j][:],
                                scalar1=th2_t[:, h : h + 1],
                                scalar2=None,
                                op0=mybir.AluOpType.is_ge,
                            )
                        else:
                            # exact on integers: arg is >= +32 or <= -32, where
                            # the sigmoid table saturates to exactly 1 / 0
                            nc.scalar.activation(
                                out=ot[:, ob : ob + NBS[t]],
                                in_=ps[j][:],
                                func=mybir.ActivationFunctionType.Sigmoid,
                                bias=th2_t[:, NHALF + h : NHALF + h + 1],
                                scale=64.0,
                            )
                        ob += NBS[t]
                    dst = out[h][:, starts[sg_start] : starts[sg_start] + ow]
                    oq.dma_start(out=dst, in_=ot[:])
    # (A weight-stationary LDWEIGHTS-elision post-pass — dropping the 237
    # duplicate per-matmul InstLdweights within spans — was tried and
    # measured ~0.5us WORSE: move_matmul_waits_to_ldweights then piles all
    # tiles' waits onto the one shared LDW, serializing each pass start on
    # its slowest dependency. The ~440ns stall every ~11us is config- and
    # instruction-count-independent (refresh-like), so fewer instructions
    # buy nothing there either.)
    nc.finalize()
    return nc


def kernel(x, weight, bias, sign):
    global LAST_RESULT
    from concourse.bass_utils import run_bass_kernel_spmd

    if "nc" not in _CACHE:
        _CACHE["nc"] = _build()
    nc = _CACHE["nc"]

    sign_v = np.asarray(sign, dtype=np.float32).reshape(COUT)
    wsig = np.asarray(weight, dtype=np.float32) * sign_v[:, None, None, None]
    # wsig[cout, cin, kh, kw] -> pairs [cin, pair, 2, cout]
    wtp_host = np.zeros((CIN, NPAIR, 2, COUT), dtype=np.float32)
    for kw in range(KW):  # pairs 0..2: (kh0, kw), (kh1, kw)
        wtp_host[:, kw, 0] = wsig[:, :, 0, kw].T
        wtp_host[:, kw, 1] = wsig[:, :, 1, kw].T
    wtp_host[:, 3, 0] = wsig[:, :, 2, 0].T  # pair 3: (kh2,kw0),(kh2,kw2)
    wtp_host[:, 3, 1] = wsig[:, :, 2, 2].T
    wtp_host[:, 4, 1] = wsig[:, :, 2, 1].T  # pair 4: (zeros, (kh2,kw1))
    wtp_host = wtp_host.reshape(CIN, NPAIR * 2 * COUT).astype(FP8)
    th_host = np.ascontiguousarray(
        (-sign_v * np.asarray(bias, dtype=np.float32)).reshape(NHALF, 128).T
    ).astype(np.float32)
    th2_host = np.concatenate(
        [th_host, (32.0 - 64.0 * th_host)], axis=1
    ).astype(np.float32)

    x = np.asarray(x, dtype=np.float32)
    in_maps = []
    for c in range(N_CORES):
        xw_host = np.zeros((CIN, XWTOT), dtype=FP8)
        xw_host[:, W4_OFF : W4_OFF + 2 * COUT] = wtp_host[:, 8 * COUT :]
        xw_host[:, W0_OFF : W0_OFF + 2 * COUT] = wtp_host[:, : 2 * COUT]
        xw_host[:, WR_OFF:] = wtp_host[:, 2 * COUT : 8 * COUT]
        xpad = xw_host[:, XS_OFF:WR_OFF].reshape(CIN, ROWS_T, WS)
        for i in range(IMGS):
            r0 = 1 + i * (H + 1)
            xpad[:, r0 : r0 + H, :W] = x[c * IMGS + i]
        in_maps.append({"xw": xw_host, "th2": th2_host})

    res = run_bass_kernel_spmd(nc, in_maps, core_ids=list(range(N_CORES)))
    LAST_RESULT = res
    # strip stored junk: out[h, co, j], j = (i*65 + r)*65 + c for valid r<64,
    # c<64 (junk at c=64 and in the 3 separator rows)
    full = np.empty((N, COUT, H, W), dtype=np.float32)
    pad1 = np.zeros((NHALF, 128, 1), dtype=FP8)
    for c, r in enumerate(res.results):
        v = np.concatenate([r["out"], pad1], axis=-1)
        v = v.reshape(NHALF, 128, ROWS_T - 2, WS)
        for i in range(IMGS):
            blk = v[:, :, i * (H + 1) : i * (H + 1) + H, :W]
            full[c * IMGS + i] = blk.reshape(COUT, H, W).astype(np.float32)
    return np.ascontiguousarray(full)

